# revision 12
# baseline (speedup 1.0000x reference)
"""Trainium2 kernel for nn_LocalSpectralAdapter.

Math: the reference rfft/irfft only modifies 16 frequency bins, so
  out = x + irfft(sparse delta-spectrum)
which is a rank-32 DFT analysis + rank-64 weighted synthesis:

  P  = F4.T @ x_b            [128, 512]  (Xr/Xi of the 16 bins, laid out twice
                                          in two different row orders)
  TT = P * G12               [128, 512]  (complex gain application, one
                                          elementwise mult; signs folded in)
  y  = x_b + Ginv2.T @ TT    [1024, 512] (crossfade weights ew/(1-ew) and the
                                          2/T irfft scale folded into Ginv2)

B=64 is sharded 8 ways across cores (pure data parallel, 8 batch/core).

The kernel is HBM-stream bound (16.8 MB in + 16.8 MB out per core), so the
schedule is built around keeping the SDMA engines fed. All 8 x loads go in
batch order on the Sync HWDGE ring, issued eagerly up front (bufs=8, no
waits ever ahead of them) so loads always land before the strictly-ordered
PE stream needs them; constants and all y stores ride the Scalar HWDGE
ring (stores issue per chunk-pair as the DVE residual add finishes).
Splitting loads across both rings was measured WORSE (115us vs 97us): odd
batches then arrive late, the serial b0..b7 PE order stalls, and both
rings end up starved mid-run.

x / f4 / ginv2 are declared float32r in DRAM: float32r is stored as the
same 4 bytes as float32, so no cast (and no SWDGE) is needed anywhere and
the BIR fp32r-rounding verifier is satisfied structurally. The PE simply
streams the raw f32 bits in 1-row/cycle f32r mode (it uses the top
mantissa bits; the sub-2^-14 difference vs pre-rounded operands is
invisible at the 2e-2 tolerance), and the residual add reuses the exact
f32 bits of the same tile.

Build notes: the module is built with bacc.Bacc and nc.compile() — TPB
instructions carry a single hardware sync-wait slot, and bacc's
generate_event_semaphores pass is what legalizes the multi-wait sync_info
Tile emits (raw bass.Bass -> walrus fails codegen with "Too many sync wait
commands").
"""

import numpy as np

_T = 1024
_V = 512
_B = 64
_NCORES = 8
_BPC = _B // _NCORES  # batch per core
_NCHUNK = _T // 128  # 8 t-chunks of 128
_BINS = np.array([1, 2, 3, 4, 5, 6, 7, 8, 12, 16, 24, 32, 48, 64, 96, 128])
_FADE_START = 487
_FADE_END = 537


def _static_transforms():
    """F4 [128,1024] (forward lhsT chunks) and Ginv2 [128,1024] (inverse lhsT),
    both independent of the gain inputs."""
    t = np.arange(_T, dtype=np.float64)
    w = 2.0 * np.pi * np.outer(t, _BINS) / _T  # [1024, 16]
    C = np.cos(w)
    S = np.sin(w)

    # Forward: PSUM rows = [Xr, Xi, Xr, Xi | Xi, Xr, Xi, Xr] blocks of 16.
    F4 = np.concatenate([C, -S, C, -S, -S, C, -S, C], axis=1)  # [1024, 128]
    # SBUF partition p holds the contiguous t-range [8p, 8p+8) (so each DMA
    # partition line is one 16KB contiguous DRAM run); matmul chunk q uses
    # t = 8p + q, i.e. lhsT chunk q at f4_dram[:, 128q:128(q+1)] with
    # f4_dram[p, 128q + m] = F4[8p + q, m].
    f4_dram = np.ascontiguousarray(
        F4.reshape(128, _NCHUNK * 128)
    ).astype(np.float32)

    fade = 1.0 - (t - _FADE_START) / (_FADE_END - _FADE_START)
    ew = np.where(t < _FADE_START, 1.0, np.where(t < _FADE_END, fade, 0.0))

    s = 2.0 / _T
    Ginv = np.concatenate(
        [s * ew * C.T, -s * ew * S.T, s * (1.0 - ew) * C.T, -s * (1.0 - ew) * S.T],
        axis=0,
    )  # [64, 1024] channels x t
    Ginv2 = np.concatenate([Ginv, Ginv], axis=0)  # [128ch, 1024t]
    # inverse lhsT chunk q: ginv2_dram[ch, 128q + p] = Ginv2[ch, 8p + q]
    ginv2_dram = np.ascontiguousarray(
        Ginv2.reshape(128, 128, _NCHUNK).transpose(0, 2, 1).reshape(128, _T)
    ).astype(np.float32)
    return f4_dram, ginv2_dram


def _gain_matrix(ger, gei, glr, gli):
    """G12 [128,512]: per-channel gain factors aligned with the PSUM row order,
    with the +/- signs of the complex multiply folded in."""
    return np.ascontiguousarray(
        np.concatenate(
            [ger.T, ger.T, glr.T, glr.T, -gei.T, gei.T, -gli.T, gli.T], axis=0
        )
    ).astype(np.float32)


_CACHED_NC = None


def _build_bass():
    global _CACHED_NC
    if _CACHED_NC is not None:
        return _CACHED_NC

    import concourse.mybir as mybir
    from concourse import bacc
    from concourse.tile import TileContext

    f32 = mybir.dt.float32
    f32r = mybir.dt.float32r
    nc = bacc.Bacc("TRN2", target_bir_lowering=False, debug=False)

    x = nc.dram_tensor("x", [_BPC, _T, _V], f32r, kind="ExternalInput").ap()
    f4 = nc.dram_tensor("f4", [128, _NCHUNK * 128], f32r, kind="ExternalInput").ap()
    ginv2 = nc.dram_tensor("ginv2", [128, _T], f32r, kind="ExternalInput").ap()
    g12 = nc.dram_tensor("g12", [128, _V], f32, kind="ExternalInput").ap()
    y = nc.dram_tensor("y", [_BPC, _T, _V], f32, kind="ExternalOutput").ap()

    with TileContext(nc) as tc:
        with (
            tc.tile_pool(name="const", bufs=1) as cpool,
            tc.tile_pool(name="xin", bufs=_BPC) as xpool,
            tc.tile_pool(name="yout", bufs=12) as ypool,
            tc.tile_pool(name="coef", bufs=2) as ttpool,
            tc.tile_pool(name="pfwd", bufs=2, space="PSUM") as ppool,
            tc.tile_pool(name="pinv", bufs=3, space="PSUM") as qpool,
        ):
            # Constants on the Scalar ring (idle until stores begin), batch
            # loads alternating Sync/Scalar so both HWDGE rings stream from
            # the first microsecond.
            f4r = cpool.tile([128, _NCHUNK * 128], f32r)
            nc.scalar.dma_start(out=f4r[:], in_=f4[:])
            ginv2r = cpool.tile([128, _T], f32r)
            nc.scalar.dma_start(out=ginv2r[:], in_=ginv2[:])
            g12sb = cpool.tile([128, _V], f32)
            nc.scalar.dma_start(out=g12sb[:], in_=g12[:])

            xsbs = []
            for b in range(_BPC):
                xsb = xpool.tile([128, _NCHUNK * _V], f32r, tag="xsb", name="xsb")
                nc.sync.dma_start(
                    out=xsb[:], in_=x[b].rearrange("(p q) v -> p (q v)", p=128)
                )
                xsbs.append(xsb)

            for b in range(_BPC):
                xsb = xsbs[b]
                xr = xsb

                # Forward DFT at the 16 bins, accumulated over the 8 t-chunks.
                P = ppool.tile([128, _V], f32)
                for c in range(_NCHUNK):
                    nc.tensor.matmul(
                        P[:],
                        lhsT=f4r[:, c * 128 : (c + 1) * 128],
                        rhs=xr[:, c * _V : (c + 1) * _V],
                        start=(c == 0),
                        stop=(c == _NCHUNK - 1),
                    )

                # Complex gain application: one elementwise multiply; the DVE
                # output stage rounds to f32r for the synthesis matmul.
                tt = ttpool.tile([128, _V], f32r)
                nc.vector.tensor_mul(tt[:], P[:], g12sb[:])

                # Weighted synthesis (chunk pairs into one 2-bank PSUM tile),
                # exact fp32 residual add on DVE; each finished pair goes out
                # immediately on the Scalar HWDGE ring.
                yv = y[b].rearrange("(p q) v -> p (q v)", p=128)
                for c2 in range(_NCHUNK // 2):
                    Q = qpool.tile([128, 2 * _V], f32)
                    for h in range(2):
                        c = 2 * c2 + h
                        nc.tensor.matmul(
                            Q[:, h * _V : (h + 1) * _V],
                            lhsT=ginv2r[:, c * 128 : (c + 1) * 128],
                            rhs=tt[:],
                            start=True,
                            stop=True,
                        )
                    ypair = ypool.tile([128, 2 * _V], f32, tag="ypair")
                    nc.vector.tensor_add(
                        ypair[:],
                        Q[:],
                        xsb[:, 2 * c2 * _V : (2 * c2 + 2) * _V],
                    )
                    nc.scalar.dma_start(
                        out=yv[:, 2 * c2 * _V : (2 * c2 + 2) * _V],
                        in_=ypair[:],
                    )

    nc.compile()
    _CACHED_NC = nc
    return nc


def _run(x, g_early_real, g_early_imag, g_late_real, g_late_imag, **spmd_kwargs):
    """Shard inputs, run the Bass kernel on 8 cores, return BassKernelResults."""
    from concourse.bass_utils import run_bass_kernel_spmd

    g_early_real = np.asarray(g_early_real, dtype=np.float32)
    g_early_imag = np.asarray(g_early_imag, dtype=np.float32)
    g_late_real = np.asarray(g_late_real, dtype=np.float32)
    g_late_imag = np.asarray(g_late_imag, dtype=np.float32)
    f4_dram, ginv2_dram = _static_transforms()
    g12_dram = _gain_matrix(g_early_real, g_early_imag, g_late_real, g_late_imag)

    x = np.ascontiguousarray(x, dtype=np.float32)
    nc = _build_bass()

    in_maps = [
        {
            "x": x[i * _BPC : (i + 1) * _BPC],
            "f4": f4_dram,
            "ginv2": ginv2_dram,
            "g12": g12_dram,
        }
        for i in range(_NCORES)
    ]
    return run_bass_kernel_spmd(
        nc, in_maps, core_ids=list(range(_NCORES)), **spmd_kwargs
    )


def kernel(x, g_early_real, g_early_imag, g_late_real, g_late_imag):
    import time

    last = None
    for _attempt in range(3):
        try:
            res = _run(x, g_early_real, g_early_imag, g_late_real, g_late_imag)
            return np.concatenate([r["y"] for r in res.results], axis=0)
        except Exception as e:
            # The axon-tunneled NeuronCores occasionally report a transient
            # NRT_EXEC_UNIT_UNRECOVERABLE right after a prior heavy run;
            # a short backoff and retry clears it.
            last = e
            msg = str(e)
            if "UNRECOVER" in msg or "UNAVAILABLE" in msg:
                time.sleep(5.0)
                continue
            raise
    raise last


# revision 16
# speedup vs baseline: 1.0347x; 1.0347x over previous
"""Trainium2 kernel for nn_LocalSpectralAdapter.

Math: the reference rfft/irfft only modifies 16 frequency bins, so
  out = x + irfft(sparse delta-spectrum)
which is a rank-32 DFT analysis + rank-64 weighted synthesis:

  P  = F4.T @ x_b            [128, 512]  (Xr/Xi of the 16 bins, laid out twice
                                          in two different row orders)
  TT = P * G12               [128, 512]  (complex gain application, one
                                          elementwise mult; signs folded in)
  y  = x_b + Ginv2.T @ TT    [1024, 512] (crossfade weights ew/(1-ew) and the
                                          2/T irfft scale folded into Ginv2)

B=64 is sharded 8 ways across cores (pure data parallel, 8 batch/core).

The kernel is HBM-stream bound (16.8 MB in + 16.8 MB out per core), so the
schedule is built around keeping the SDMA engines fed. All 8 x loads go in
batch order as SWDGE cast-DMAs (f32 -> f32r) on the GpSimd ring, issued
eagerly up front (bufs=8, no waits ever ahead of them) so loads always
land before the strictly-ordered PE stream needs them. Constants ride the
Sync HWDGE ring at t=0; y stores alternate between the Scalar and Sync
HWDGE rings per chunk-pair as the DVE residual add finishes (two rings
drain ~435 GB/s vs ~420 for one).

Two measured traps shape this layout: (1) loads and stores must live in
different DGE classes — SWDGE completions use the DMASW semaphore lanes,
HWDGE the DMAHW lanes, and sharing one class makes eager loads cross-wait
on store completions (+19us); (2) loads must arrive in the b0..b7 compute
order — round-robining them across rings stalls the serial PE stream.

The SWDGE "cast" is a plain bit copy (f32r is stored as the same 4 bytes;
the PE itself truncates mantissas in f32r streaming mode) but it is the
cheapest producer that satisfies the BIR fp32r-rounding verifier, and the
residual add reuses the same tile's exact f32 bits.

Build notes: the module is built with bacc.Bacc and nc.compile() — TPB
instructions carry a single hardware sync-wait slot, and bacc's
generate_event_semaphores pass is what legalizes the multi-wait sync_info
Tile emits (raw bass.Bass -> walrus fails codegen with "Too many sync wait
commands").
"""

import numpy as np

_T = 1024
_V = 512
_B = 64
_NCORES = 8
_BPC = _B // _NCORES  # batch per core
_NCHUNK = _T // 128  # 8 t-chunks of 128
_BINS = np.array([1, 2, 3, 4, 5, 6, 7, 8, 12, 16, 24, 32, 48, 64, 96, 128])
_FADE_START = 487
_FADE_END = 537


def _static_transforms():
    """F4 [128,1024] (forward lhsT chunks) and Ginv2 [128,1024] (inverse lhsT),
    both independent of the gain inputs."""
    t = np.arange(_T, dtype=np.float64)
    w = 2.0 * np.pi * np.outer(t, _BINS) / _T  # [1024, 16]
    C = np.cos(w)
    S = np.sin(w)

    # Forward: PSUM rows = [Xr, Xi, Xr, Xi | Xi, Xr, Xi, Xr] blocks of 16.
    F4 = np.concatenate([C, -S, C, -S, -S, C, -S, C], axis=1)  # [1024, 128]
    # SBUF partition p holds the contiguous t-range [8p, 8p+8) (so each DMA
    # partition line is one 16KB contiguous DRAM run); matmul chunk q uses
    # t = 8p + q, i.e. lhsT chunk q at f4_dram[:, 128q:128(q+1)] with
    # f4_dram[p, 128q + m] = F4[8p + q, m].
    f4_dram = np.ascontiguousarray(
        F4.reshape(128, _NCHUNK * 128)
    ).astype(np.float32)

    fade = 1.0 - (t - _FADE_START) / (_FADE_END - _FADE_START)
    ew = np.where(t < _FADE_START, 1.0, np.where(t < _FADE_END, fade, 0.0))

    s = 2.0 / _T
    Ginv = np.concatenate(
        [s * ew * C.T, -s * ew * S.T, s * (1.0 - ew) * C.T, -s * (1.0 - ew) * S.T],
        axis=0,
    )  # [64, 1024] channels x t
    Ginv2 = np.concatenate([Ginv, Ginv], axis=0)  # [128ch, 1024t]
    # inverse lhsT chunk q: ginv2_dram[ch, 128q + p] = Ginv2[ch, 8p + q]
    ginv2_dram = np.ascontiguousarray(
        Ginv2.reshape(128, 128, _NCHUNK).transpose(0, 2, 1).reshape(128, _T)
    ).astype(np.float32)
    return f4_dram, ginv2_dram


def _gain_matrix(ger, gei, glr, gli):
    """G12 [128,512]: per-channel gain factors aligned with the PSUM row order,
    with the +/- signs of the complex multiply folded in."""
    return np.ascontiguousarray(
        np.concatenate(
            [ger.T, ger.T, glr.T, glr.T, -gei.T, gei.T, -gli.T, gli.T], axis=0
        )
    ).astype(np.float32)


_CACHED_NC = None


def _build_bass():
    global _CACHED_NC
    if _CACHED_NC is not None:
        return _CACHED_NC

    import concourse.mybir as mybir
    from concourse import bacc
    from concourse.tile import TileContext

    f32 = mybir.dt.float32
    f32r = mybir.dt.float32r
    nc = bacc.Bacc("TRN2", target_bir_lowering=False, debug=False)

    x = nc.dram_tensor("x", [_BPC, _T, _V], f32, kind="ExternalInput").ap()
    f4 = nc.dram_tensor("f4", [128, _NCHUNK * 128], f32r, kind="ExternalInput").ap()
    ginv2 = nc.dram_tensor("ginv2", [128, _T], f32r, kind="ExternalInput").ap()
    g12 = nc.dram_tensor("g12", [128, _V], f32, kind="ExternalInput").ap()
    y = nc.dram_tensor("y", [_BPC, _T, _V], f32, kind="ExternalOutput").ap()

    with TileContext(nc) as tc:
        with (
            tc.tile_pool(name="const", bufs=1) as cpool,
            tc.tile_pool(name="xin", bufs=_BPC) as xpool,
            tc.tile_pool(name="yout", bufs=12) as ypool,
            tc.tile_pool(name="coef", bufs=2) as ttpool,
            tc.tile_pool(name="pfwd", bufs=2, space="PSUM") as ppool,
            tc.tile_pool(name="pinv", bufs=3, space="PSUM") as qpool,
        ):
            # Constants on the Sync HWDGE ring (idle at start), x loads as
            # SWDGE cast-DMAs on the GpSimd ring. The split is deliberate:
            # SWDGE completions use the 8 DMASW semaphore lanes while HWDGE
            # uses the 8 DMAHW lanes, so the eager load stream never
            # cross-waits on store completions (putting loads on HWDGE was
            # measured 19us slower purely from that lane sharing).
            f4r = cpool.tile([128, _NCHUNK * 128], f32r)
            nc.sync.dma_start(out=f4r[:], in_=f4[:])
            ginv2r = cpool.tile([128, _T], f32r)
            nc.sync.dma_start(out=ginv2r[:], in_=ginv2[:])
            g12sb = cpool.tile([128, _V], f32)
            nc.sync.dma_start(out=g12sb[:], in_=g12[:])

            xsbs = []
            for b in range(_BPC):
                xsb = xpool.tile([128, _NCHUNK * _V], f32r, tag="xsb", name="xsb")
                nc.gpsimd.dma_start(
                    out=xsb[:], in_=x[b].rearrange("(p q) v -> p (q v)", p=128)
                )
                xsbs.append(xsb)

            for b in range(_BPC):
                xsb = xsbs[b]
                xr = xsb

                # Forward DFT at the 16 bins, accumulated over the 8 t-chunks.
                P = ppool.tile([128, _V], f32)
                for c in range(_NCHUNK):
                    nc.tensor.matmul(
                        P[:],
                        lhsT=f4r[:, c * 128 : (c + 1) * 128],
                        rhs=xr[:, c * _V : (c + 1) * _V],
                        start=(c == 0),
                        stop=(c == _NCHUNK - 1),
                    )

                # Complex gain application: one elementwise multiply; the DVE
                # output stage rounds to f32r for the synthesis matmul.
                tt = ttpool.tile([128, _V], f32r)
                nc.vector.tensor_mul(tt[:], P[:], g12sb[:])

                # Weighted synthesis (chunk pairs into one 2-bank PSUM tile),
                # exact fp32 residual add on DVE; each finished pair goes out
                # immediately on the Scalar HWDGE ring.
                yv = y[b].rearrange("(p q) v -> p (q v)", p=128)
                for c2 in range(_NCHUNK // 2):
                    Q = qpool.tile([128, 2 * _V], f32)
                    for h in range(2):
                        c = 2 * c2 + h
                        nc.tensor.matmul(
                            Q[:, h * _V : (h + 1) * _V],
                            lhsT=ginv2r[:, c * 128 : (c + 1) * 128],
                            rhs=tt[:],
                            start=True,
                            stop=True,
                        )
                    ypair = ypool.tile([128, 2 * _V], f32, tag="ypair")
                    nc.vector.tensor_add(
                        ypair[:],
                        Q[:],
                        xsb[:, 2 * c2 * _V : (2 * c2 + 2) * _V],
                    )
                    seng = nc.scalar if c2 % 2 == 0 else nc.sync
                    seng.dma_start(
                        out=yv[:, 2 * c2 * _V : (2 * c2 + 2) * _V],
                        in_=ypair[:],
                    )

    nc.compile()
    _CACHED_NC = nc
    return nc


def _run(x, g_early_real, g_early_imag, g_late_real, g_late_imag, **spmd_kwargs):
    """Shard inputs, run the Bass kernel on 8 cores, return BassKernelResults."""
    from concourse.bass_utils import run_bass_kernel_spmd

    g_early_real = np.asarray(g_early_real, dtype=np.float32)
    g_early_imag = np.asarray(g_early_imag, dtype=np.float32)
    g_late_real = np.asarray(g_late_real, dtype=np.float32)
    g_late_imag = np.asarray(g_late_imag, dtype=np.float32)
    f4_dram, ginv2_dram = _static_transforms()
    g12_dram = _gain_matrix(g_early_real, g_early_imag, g_late_real, g_late_imag)

    x = np.ascontiguousarray(x, dtype=np.float32)
    nc = _build_bass()

    in_maps = [
        {
            "x": x[i * _BPC : (i + 1) * _BPC],
            "f4": f4_dram,
            "ginv2": ginv2_dram,
            "g12": g12_dram,
        }
        for i in range(_NCORES)
    ]
    return run_bass_kernel_spmd(
        nc, in_maps, core_ids=list(range(_NCORES)), **spmd_kwargs
    )


def kernel(x, g_early_real, g_early_imag, g_late_real, g_late_imag):
    import time

    last = None
    for _attempt in range(3):
        try:
            res = _run(x, g_early_real, g_early_imag, g_late_real, g_late_imag)
            return np.concatenate([r["y"] for r in res.results], axis=0)
        except Exception as e:
            # The axon-tunneled NeuronCores occasionally report a transient
            # NRT_EXEC_UNIT_UNRECOVERABLE right after a prior heavy run;
            # a short backoff and retry clears it.
            last = e
            msg = str(e)
            if "UNRECOVER" in msg or "UNAVAILABLE" in msg:
                time.sleep(5.0)
                continue
            raise
    raise last


# revision 20
# speedup vs baseline: 1.1995x; 1.1593x over previous
"""Trainium2 kernel for nn_LocalSpectralAdapter.

Math: the reference rfft/irfft only modifies 16 frequency bins, so
  out = x + irfft(sparse delta-spectrum)
which is a rank-32 DFT analysis + rank-64 weighted synthesis:

  P  = F4.T @ x_b            [128, 512]  (Xr/Xi of the 16 bins, laid out twice
                                          in two different row orders)
  TT = P * G12               [128, 512]  (complex gain application, one
                                          elementwise mult; signs folded in)
  y  = x_b + Ginv2.T @ TT    [1024, 512] (crossfade weights ew/(1-ew) and the
                                          2/T irfft scale folded into Ginv2)

B=64 is sharded 8 ways across cores (pure data parallel, 8 batch/core).

The kernel is HBM-stream bound (16.8 MB in + 16.8 MB out per core), so the
schedule is built around keeping the SDMA engines fed. Constants then all
8 x loads go in batch order as SWDGE cast-DMAs (f32 -> f32r) on the
GpSimd ring, issued eagerly up front (bufs=8, no waits ever ahead of
them) so loads always land before the strictly-ordered PE stream needs
them. All y stores go on the Sync HWDGE ring, issued per chunk-pair as
the DVE residual add finishes.

Measured traps that shape this exact layout (each alternative cost
+15-19us): (1) loads and stores must live in different DGE classes —
SWDGE completions use the DMASW semaphore lanes, HWDGE the DMAHW lanes,
and sharing one class makes eager loads cross-wait on store completions;
(2) loads must arrive in b0..b7 order on ONE ring — round-robining them
across rings stalls the serial PE stream; (3) stores on one HWDGE ring
drain at ~420 GB/s, splitting them across Sync+Scalar dropped to ~350;
(4) the small constant DMAs throttle the load ring far less than an HWDGE
ring (packet-granularity round-robin favors the loads' 16KB packets).

The SWDGE "cast" is a plain bit copy (f32r is stored as the same 4 bytes;
the PE itself truncates mantissas in f32r streaming mode) but it is the
cheapest producer that satisfies the BIR fp32r-rounding verifier, and the
residual add reuses the same tile's exact f32 bits.

Build notes: the module is built with bacc.Bacc and nc.compile() — TPB
instructions carry a single hardware sync-wait slot, and bacc's
generate_event_semaphores pass is what legalizes the multi-wait sync_info
Tile emits (raw bass.Bass -> walrus fails codegen with "Too many sync wait
commands").
"""

import numpy as np

_T = 1024
_V = 512
_B = 64
_NCORES = 8
_BPC = _B // _NCORES  # batch per core
_NCHUNK = _T // 128  # 8 t-chunks of 128
_BINS = np.array([1, 2, 3, 4, 5, 6, 7, 8, 12, 16, 24, 32, 48, 64, 96, 128])
_FADE_START = 487
_FADE_END = 537


def _static_transforms():
    """F4 [128,1024] (forward lhsT chunks) and Ginv2 [128,1024] (inverse lhsT),
    both independent of the gain inputs."""
    t = np.arange(_T, dtype=np.float64)
    w = 2.0 * np.pi * np.outer(t, _BINS) / _T  # [1024, 16]
    C = np.cos(w)
    S = np.sin(w)

    # Forward: PSUM rows = [Xr, Xi, Xr, Xi | Xi, Xr, Xi, Xr] blocks of 16.
    F4 = np.concatenate([C, -S, C, -S, -S, C, -S, C], axis=1)  # [1024, 128]
    # SBUF partition p holds the contiguous t-range [8p, 8p+8) (so each DMA
    # partition line is one 16KB contiguous DRAM run); matmul chunk q uses
    # t = 8p + q, i.e. lhsT chunk q at f4_dram[:, 128q:128(q+1)] with
    # f4_dram[p, 128q + m] = F4[8p + q, m].
    f4_dram = np.ascontiguousarray(
        F4.reshape(128, _NCHUNK * 128)
    ).astype(np.float32)

    fade = 1.0 - (t - _FADE_START) / (_FADE_END - _FADE_START)
    ew = np.where(t < _FADE_START, 1.0, np.where(t < _FADE_END, fade, 0.0))

    s = 2.0 / _T
    Ginv = np.concatenate(
        [s * ew * C.T, -s * ew * S.T, s * (1.0 - ew) * C.T, -s * (1.0 - ew) * S.T],
        axis=0,
    )  # [64, 1024] channels x t
    Ginv2 = np.concatenate([Ginv, Ginv], axis=0)  # [128ch, 1024t]
    # inverse lhsT chunk q: ginv2_dram[ch, 128q + p] = Ginv2[ch, 8p + q]
    ginv2_dram = np.ascontiguousarray(
        Ginv2.reshape(128, 128, _NCHUNK).transpose(0, 2, 1).reshape(128, _T)
    ).astype(np.float32)
    return f4_dram, ginv2_dram


def _gain_matrix(ger, gei, glr, gli):
    """G12 [128,512]: per-channel gain factors aligned with the PSUM row order,
    with the +/- signs of the complex multiply folded in."""
    return np.ascontiguousarray(
        np.concatenate(
            [ger.T, ger.T, glr.T, glr.T, -gei.T, gei.T, -gli.T, gli.T], axis=0
        )
    ).astype(np.float32)


_CACHED_NC = None


def _build_bass():
    global _CACHED_NC
    if _CACHED_NC is not None:
        return _CACHED_NC

    import concourse.mybir as mybir
    from concourse import bacc
    from concourse.tile import TileContext

    f32 = mybir.dt.float32
    f32r = mybir.dt.float32r
    nc = bacc.Bacc("TRN2", target_bir_lowering=False, debug=False)

    x = nc.dram_tensor("x", [_BPC, _T, _V], f32, kind="ExternalInput").ap()
    f4 = nc.dram_tensor("f4", [128, _NCHUNK * 128], f32, kind="ExternalInput").ap()
    ginv2 = nc.dram_tensor("ginv2", [128, _T], f32, kind="ExternalInput").ap()
    g12 = nc.dram_tensor("g12", [128, _V], f32, kind="ExternalInput").ap()
    y = nc.dram_tensor("y", [_BPC, _T, _V], f32, kind="ExternalOutput").ap()

    with TileContext(nc) as tc:
        with (
            tc.tile_pool(name="const", bufs=1) as cpool,
            tc.tile_pool(name="xin", bufs=_BPC) as xpool,
            tc.tile_pool(name="yout", bufs=12) as ypool,
            tc.tile_pool(name="coef", bufs=2) as ttpool,
            tc.tile_pool(name="pfwd", bufs=2, space="PSUM") as ppool,
            tc.tile_pool(name="pinv", bufs=3, space="PSUM") as qpool,
        ):
            # Constants first on the GpSimd SWDGE ring (cast f32 -> f32r in
            # the DMA datapath), then the x loads on the same ring. Keeping
            # the constants OFF the HWDGE rings matters: their small (4KB)
            # packets round-robin 1:1 against the loads' 16KB packets and
            # were measured to throttle the early load stream when issued on
            # Sync instead.
            f4r = cpool.tile([128, _NCHUNK * 128], f32r)
            nc.gpsimd.dma_start(out=f4r[:], in_=f4[:])
            ginv2r = cpool.tile([128, _T], f32r)
            nc.gpsimd.dma_start(out=ginv2r[:], in_=ginv2[:])
            g12sb = cpool.tile([128, _V], f32)
            nc.sync.dma_start(out=g12sb[:], in_=g12[:])

            xsbs = []
            for b in range(_BPC):
                xsb = xpool.tile([128, _NCHUNK * _V], f32r, tag="xsb", name="xsb")
                nc.gpsimd.dma_start(
                    out=xsb[:], in_=x[b].rearrange("(p q) v -> p (q v)", p=128)
                )
                xsbs.append(xsb)

            for b in range(_BPC):
                xsb = xsbs[b]
                xr = xsb

                # Forward DFT at the 16 bins, accumulated over the 8 t-chunks.
                P = ppool.tile([128, _V], f32)
                for c in range(_NCHUNK):
                    nc.tensor.matmul(
                        P[:],
                        lhsT=f4r[:, c * 128 : (c + 1) * 128],
                        rhs=xr[:, c * _V : (c + 1) * _V],
                        start=(c == 0),
                        stop=(c == _NCHUNK - 1),
                    )

                # Complex gain application: one elementwise multiply; the DVE
                # output stage rounds to f32r for the synthesis matmul.
                tt = ttpool.tile([128, _V], f32r)
                nc.vector.tensor_mul(tt[:], P[:], g12sb[:])

                # Weighted synthesis (chunk pairs into one 2-bank PSUM tile),
                # exact fp32 residual add on DVE; each finished pair goes out
                # immediately on the Scalar HWDGE ring.
                yv = y[b].rearrange("(p q) v -> p (q v)", p=128)
                for c2 in range(_NCHUNK // 2):
                    Q = qpool.tile([128, 2 * _V], f32)
                    for h in range(2):
                        c = 2 * c2 + h
                        nc.tensor.matmul(
                            Q[:, h * _V : (h + 1) * _V],
                            lhsT=ginv2r[:, c * 128 : (c + 1) * 128],
                            rhs=tt[:],
                            start=True,
                            stop=True,
                        )
                    ypair = ypool.tile([128, 2 * _V], f32, tag="ypair")
                    nc.vector.tensor_add(
                        ypair[:],
                        Q[:],
                        xsb[:, 2 * c2 * _V : (2 * c2 + 2) * _V],
                    )
                    nc.sync.dma_start(
                        out=yv[:, 2 * c2 * _V : (2 * c2 + 2) * _V],
                        in_=ypair[:],
                    )

    nc.compile()
    _CACHED_NC = nc
    return nc


def _run(x, g_early_real, g_early_imag, g_late_real, g_late_imag, **spmd_kwargs):
    """Shard inputs, run the Bass kernel on 8 cores, return BassKernelResults."""
    from concourse.bass_utils import run_bass_kernel_spmd

    g_early_real = np.asarray(g_early_real, dtype=np.float32)
    g_early_imag = np.asarray(g_early_imag, dtype=np.float32)
    g_late_real = np.asarray(g_late_real, dtype=np.float32)
    g_late_imag = np.asarray(g_late_imag, dtype=np.float32)
    f4_dram, ginv2_dram = _static_transforms()
    g12_dram = _gain_matrix(g_early_real, g_early_imag, g_late_real, g_late_imag)

    x = np.ascontiguousarray(x, dtype=np.float32)
    nc = _build_bass()

    in_maps = [
        {
            "x": x[i * _BPC : (i + 1) * _BPC],
            "f4": f4_dram,
            "ginv2": ginv2_dram,
            "g12": g12_dram,
        }
        for i in range(_NCORES)
    ]
    return run_bass_kernel_spmd(
        nc, in_maps, core_ids=list(range(_NCORES)), **spmd_kwargs
    )


def kernel(x, g_early_real, g_early_imag, g_late_real, g_late_imag):
    import time

    last = None
    for _attempt in range(3):
        try:
            res = _run(x, g_early_real, g_early_imag, g_late_real, g_late_imag)
            return np.concatenate([r["y"] for r in res.results], axis=0)
        except Exception as e:
            # The axon-tunneled NeuronCores occasionally report a transient
            # NRT_EXEC_UNIT_UNRECOVERABLE right after a prior heavy run;
            # a short backoff and retry clears it.
            last = e
            msg = str(e)
            if "UNRECOVER" in msg or "UNAVAILABLE" in msg:
                time.sleep(5.0)
                continue
            raise
    raise last
